# revision 28
# baseline (speedup 1.0000x reference)
"""CrossAttention Trainium2 Bass kernel.

Problem: B=2, Q=S=2048, D=1024, H=16 heads, A=64 head_dim.
  q = (iQ @ Wq)   -> [B,H,Q,A]
  k,v = iK @ Wkv  -> [B,H,S,A] each
  scores = q k^T / 8, mask -> -1e9, softmax over S
  out = (attn @ v) @ Wo -> [B,Q,D]

Sharding: 8 cores = 2 batches x 4 head-groups (4 heads each).
Each core computes a partial [Q, D] = ctx_local @ Wo_rows(local heads);
host sums the 4 partials per batch (row-parallel Wo unshard).

Mask pruning: masked s positions contribute exactly 0 to the softmax
(exp(-1e9) == 0.0 in f32, same as the reference), so the host gathers
only the unmasked iK rows (~S/2 of them), padded to a multiple of 128
with zero K columns and -1e9 bias so padding also exps to exactly 0.

Device layout trick: everything is computed "transposed" (feature dim on
partitions) so no on-device transposes are needed:
  - host ships iQ^T, iK^T (pre-tiled [128, 8, n])
  - qT[a,q], kT[a,s] from matmul(lhsT=W, rhs=iX^T)
  - scoresT[s,q] = matmul(lhsT=kT_slice, rhs=qT)       (K=64 contraction)
  - exp via scalar activation, mask bias is a per-partition bias AP
  - V kept natural [s,a] with an appended ones column -> attn@V matmul
    also yields the softmax denominator row for free
  - ctxT normalized via reciprocal + K=1 outer-product broadcast
  - out[q,n] = matmul(lhsT=ctxT_h tile, rhs=Wo_h rows), accum over heads
All psum->sbuf copies run on DVE so the Scalar engine only does exp
(exp paces the attention phase; the PE p-state depends on keeping up).
"""

import sys
import numpy as np

for _p in ("/opt/trn_rl_repo",):
    if _p not in sys.path:
        sys.path.insert(0, _p)

import ml_dtypes

B, Q, S, D = 2, 2048, 2048, 1024
H, A = 16, 64
HG = 4            # heads per core
NCORES = 8
NEG = -1e9
MIN_NST = 9       # S tiles after mask pruning (1152 slots; count ~1024)

_cache = {}


def _build_program(nst):
    import concourse.bass as bass  # noqa
    import concourse.bacc as bacc
    import concourse.tile as tile
    from concourse import mybir

    f32 = mybir.dt.float32
    bf16 = mybir.dt.bfloat16
    EXP = mybir.ActivationFunctionType.Exp
    LN = mybir.ActivationFunctionType.Ln
    MULT = mybir.AluOpType.mult

    nc = bacc.Bacc("TRN2", target_bir_lowering=False, debug=False)

    SP = nst * 128  # padded kept-S extent
    iqt = nc.dram_tensor("iqt", [128, 8, Q], bf16, kind="ExternalInput").ap()
    ikt = nc.dram_tensor("ikt", [128, 8, SP], bf16, kind="ExternalInput").ap()
    wq = nc.dram_tensor("wq", [128, 8, 256], bf16, kind="ExternalInput").ap()
    wk = nc.dram_tensor("wk", [128, 8, 256], bf16, kind="ExternalInput").ap()
    wv = nc.dram_tensor("wv", [128, 8, 256], bf16, kind="ExternalInput").ap()
    wo = nc.dram_tensor("wo", [128, 2, D], bf16, kind="ExternalInput").ap()
    mb = nc.dram_tensor("mb", [128, nst], f32, kind="ExternalInput").ap()
    out = nc.dram_tensor("out", [128, 16, D], f32, kind="ExternalOutput").ap()

    NQT = Q // 128          # 16 q tiles
    NDT = D // 128          # 8 d tiles

    with tile.TileContext(nc) as tc:
        with (
            tc.tile_pool(name="persist", bufs=1) as persist,
            tc.tile_pool(name="expp", bufs=3) as expp,
            tc.tile_pool(name="outp", bufs=3) as outp,
            tc.tile_pool(name="srp", bufs=2) as srp,
            tc.tile_pool(name="scp", bufs=2, space="PSUM") as scp,
            tc.tile_pool(name="ctxp", bufs=1, space="PSUM") as ctxp,
            tc.tile_pool(name="bcp", bufs=1, space="PSUM") as bcp,
        ):
            # ---- persistent loads, in consumption order: small weights
            # first, then iK^T/iQ^T tiles interleaved so the KT projection
            # starts as soon as wk + the first ikt tile land.
            wk_sb = persist.tile([128, 8, 256], bf16, tag="wk")
            nc.sync.dma_start(wk_sb[:], wk[:])
            wq_sb = persist.tile([128, 8, 256], bf16, tag="wq")
            nc.sync.dma_start(wq_sb[:], wq[:])
            wv_sb = persist.tile([128, 8, 256], bf16, tag="wv")
            nc.sync.dma_start(wv_sb[:], wv[:])
            wo_sb = persist.tile([128, 2, D], bf16, tag="wo")
            nc.sync.dma_start(wo_sb[:], wo[:])
            mb_sb = persist.tile([128, nst], f32, tag="mb")
            nc.sync.dma_start(mb_sb[:], mb[:])
            iqt_sb = persist.tile([128, 8, Q], bf16, tag="iqt")
            ikt_sb = persist.tile([128, 8, SP], bf16, tag="ikt")
            for dt_i in range(8):
                hk = SP // 2
                nc.sync.dma_start(ikt_sb[:, dt_i, :hk], ikt[:, dt_i, :hk])
                nc.sync.dma_start(ikt_sb[:, dt_i, hk:], ikt[:, dt_i, hk:])
                nc.sync.dma_start(iqt_sb[:, dt_i, :1024], iqt[:, dt_i, :1024])
                nc.sync.dma_start(iqt_sb[:, dt_i, 1024:], iqt[:, dt_i, 1024:])

            qt_sb = persist.tile([128, 2, Q], bf16, tag="qt")
            kt_sb = persist.tile([128, 2, SP], bf16, tag="kt")
            # V padded to 128 cols (fast weight load wants full-width lhsT);
            # col 64 = ones (softmax denominator row), cols 65.. = zeros.
            v_sb = persist.tile([128, nst, HG, 128], bf16, tag="v")
            nc.vector.memset(v_sb[:], 0.0)
            nc.vector.memset(v_sb[:, :, :, 64:65], 1.0)
            # two heads packed per 128-partition tile for the Wo matmul;
            # separate tiles per q-chunk so Wo reads of chunk 0 don't
            # falsely depend on chunk 1 writes.
            ctxn = [
                [
                    persist.tile(
                        [128, 1024], bf16, tag=f"ctxn{qc}{t}", name=f"ctxn{qc}{t}"
                    )
                    for t in range(2)
                ]
                for qc in range(2)
            ]

            # ---- projections: kT [a, s] first (its inputs land first) ----
            for wsb, xsb, osb, nf in (
                (wk_sb, ikt_sb, kt_sb, SP),
                (wq_sb, iqt_sb, qt_sb, Q),
            ):
                nchunk = (nf + 1023) // 1024
                for at in range(2):          # 128-wide slab of the 256 head cols
                    for qc in range(nchunk):  # 1024-wide output chunk
                        w = min(1024, nf - qc * 1024)
                        ps = scp.tile([128, 1024], mybir.dt.float32, tag="mm")
                        for c in range(0, w, 512):
                            cw = min(512, w - c)
                            for dt_i in range(NDT):
                                nc.tensor.matmul(
                                    ps[:, c:c + cw],
                                    lhsT=wsb[:, dt_i, at * 128:(at + 1) * 128],
                                    rhs=xsb[:, dt_i, qc * 1024 + c:qc * 1024 + c + cw],
                                    start=(dt_i == 0),
                                    stop=(dt_i == NDT - 1),
                                )
                        nc.vector.tensor_copy(
                            out=osb[:, at, qc * 1024:qc * 1024 + w], in_=ps[:, :w]
                        )

            # ---- V projection: natural [s, a] per head (+ ones col kept) ----
            for st in range(nst):
                ps = scp.tile([128, HG, 64], mybir.dt.float32, tag="mm")
                for dt_i in range(NDT):
                    nc.tensor.matmul(
                        ps[:],
                        lhsT=ikt_sb[:, dt_i, st * 128:(st + 1) * 128],
                        rhs=wv_sb[:, dt_i, :],
                        start=(dt_i == 0),
                        stop=(dt_i == NDT - 1),
                    )
                nc.vector.tensor_copy(out=v_sb[:, st, :, 0:64], in_=ps[:])

            # ---- attention per (q-chunk, head) + interleaved Wo ----
            def emit_wo(qt):
                ps = scp.tile([128, 1024], mybir.dt.float32, tag="mm")
                for c in range(2):
                    for t in range(2):
                        nc.tensor.matmul(
                            ps[:, c * 512:(c + 1) * 512],
                            lhsT=ctxn[qt // 8][t][:, (qt % 8) * 128:
                                                  (qt % 8 + 1) * 128],
                            rhs=wo_sb[:, t, c * 512:(c + 1) * 512],
                            start=(t == 0),
                            stop=(t == 1),
                        )
                ob = outp.tile([128, 1024], mybir.dt.float32, tag="ob")
                nc.vector.tensor_copy(out=ob[:], in_=ps[:])
                nc.sync.dma_start(out[:, qt, :], ob[:])

            for qc in range(2):
                q0 = qc * 1024
                for h in range(HG):
                    po = (h % 2) * 64
                    ti = h // 2
                    ctx = ctxp.tile([128, 1024], mybir.dt.float32, tag="ctx")
                    for st in range(nst):
                        sc = scp.tile([128, 1024], mybir.dt.float32, tag="mm")
                        for c in range(2):
                            nc.tensor.matmul(
                                sc[:, c * 512:(c + 1) * 512],
                                lhsT=kt_sb[po:po + 64, ti, st * 128:(st + 1) * 128],
                                rhs=qt_sb[po:po + 64, ti,
                                          q0 + c * 512:q0 + (c + 1) * 512],
                                start=True,
                                stop=True,
                            )
                        ex = expp.tile([128, 1024], bf16, tag="exp")
                        nc.scalar.activation(
                            out=ex[:], in_=sc[:], func=EXP,
                            bias=mb_sb[:, st:st + 1], scale=0.125,
                        )
                        for c in range(2):
                            nc.tensor.matmul(
                                ctx[:, c * 512:(c + 1) * 512],
                                lhsT=v_sb[:, st, h, :],
                                rhs=ex[:, c * 512:(c + 1) * 512],
                                start=(st == 0),
                                stop=(st == nst - 1),
                            )
                    # Copy ctx out of PSUM on ACT right after this head's
                    # last exp (hidden behind the next head's scores), so
                    # the PSUM bank frees immediately. The reciprocal runs
                    # on DVE (slow but fully overlapped), broadcast on
                    # GpSimd, multiply on DVE into the packed ctxn tile.
                    ctxu = srp.tile([65, 1024], mybir.dt.float32, tag="ctxu")
                    nc.scalar.copy(out=ctxu[:], in_=ctx[:65, :])
                    recip = srp.tile([1, 1024], mybir.dt.float32, tag="recip")
                    if qc == 1 and h == HG - 1:
                        # last head: ln->exp on ACT (1.5us) instead of the
                        # 6.5us DVE reciprocal, to shorten the kernel tail
                        lnd = srp.tile([1, 1024], mybir.dt.float32, tag="lnd")
                        nc.scalar.activation(out=lnd[:], in_=ctxu[64:65, :],
                                             func=LN)
                        nc.scalar.activation(out=recip[:], in_=lnd[:],
                                             func=EXP, scale=-1.0)
                    else:
                        nc.vector.reciprocal(recip[:], ctxu[64:65, :])
                    bcd = srp.tile([64, 1024], mybir.dt.float32, tag="bcd")
                    nc.gpsimd.partition_broadcast(bcd[:], recip[:])
                    nc.vector.tensor_tensor(
                        ctxn[qc][ti][po:po + 64, :],
                        ctxu[0:64, :], bcd[:], MULT,
                    )
                    # interleave the previous q-chunk's Wo tiles as PE filler
                    if qc == 1:
                        for qt in (h * 2, h * 2 + 1):
                            emit_wo(qt)
            for qt in range(8, 16):
                emit_wo(qt)

    nc.compile()
    return nc


def _get_program(nst):
    if nst not in _cache:
        _cache[nst] = _build_program(nst)
    return _cache[nst]


def _prep_inputs(iQ, iK, mask, Wq, Wkv, Wo):
    """Build the 8 per-core input maps (host-side shard + prune + cast)."""
    bf = ml_dtypes.bfloat16
    iQ = np.asarray(iQ, dtype=np.float32)
    iK = np.asarray(iK, dtype=np.float32)
    mask = np.asarray(mask)
    Wq = np.asarray(Wq, dtype=np.float32)
    Wkv = np.asarray(Wkv, dtype=np.float32)
    Wo = np.asarray(Wo, dtype=np.float32)

    def tile_kxn(a):  # [K=1024, N] -> [128, K/128, N]
        K, N = a.shape
        return np.ascontiguousarray(
            a.reshape(K // 128, 128, N).transpose(1, 0, 2)
        )

    kept = [np.flatnonzero(~mask[b, 0]) for b in range(B)]
    nst = max(MIN_NST, max((len(k) + 127) // 128 for k in kept))
    SP = nst * 128

    per_b = {}
    for b in range(B):
        nk = len(kept[b])
        ikt_full = np.zeros((1024, SP), dtype=np.float32)
        ikt_full[:, :nk] = iK[b][kept[b], :].T
        bias = np.full(SP, np.float32(NEG), dtype=np.float32)
        bias[:nk] = 0.0
        per_b[b] = {
            "iqt": tile_kxn(iQ[b].T).astype(bf),
            "ikt": tile_kxn(ikt_full).astype(bf),
            "mb": np.ascontiguousarray(bias.reshape(nst, 128).T),
        }
    in_maps = []
    for c in range(NCORES):
        b, g = divmod(c, NCORES // B)
        cols = slice(g * 256, (g + 1) * 256)
        wo_g = Wo[g * 256:(g + 1) * 256, :]          # [256, 1024]
        in_maps.append({
            "iqt": per_b[b]["iqt"],
            "ikt": per_b[b]["ikt"],
            "mb": per_b[b]["mb"],
            "wq": tile_kxn(Wq[:, cols]).astype(bf),
            "wk": tile_kxn(Wkv[:, cols]).astype(bf),
            "wv": tile_kxn(Wkv[:, 1024 + g * 256:1024 + (g + 1) * 256]).astype(bf),
            "wo": np.ascontiguousarray(
                wo_g.reshape(2, 128, D).transpose(1, 0, 2)
            ).astype(bf),
        })
    return in_maps, nst


def _run(inputs, trace=False):
    from concourse.bass_utils import run_bass_kernel_spmd

    in_maps, nst = _prep_inputs(**inputs)
    nc = _get_program(nst)
    res = run_bass_kernel_spmd(
        nc, in_maps, list(range(NCORES)), trace=trace
    )
    outs = []
    for b in range(B):
        acc = None
        for g in range(NCORES // B):
            o = np.asarray(
                res.results[b * (NCORES // B) + g]["out"], dtype=np.float32
            )
            acc = o if acc is None else acc + o
        # [128, 16, 1024] -> [2048, 1024]
        outs.append(acc.transpose(1, 0, 2).reshape(Q, D))
    return np.stack(outs), res


def kernel(**inputs):
    out, _ = _run(inputs, trace=False)
    return out


# revision 29
# speedup vs baseline: 1.0216x; 1.0216x over previous
"""CrossAttention Trainium2 Bass kernel.

Problem: B=2, Q=S=2048, D=1024, H=16 heads, A=64 head_dim.
  q = (iQ @ Wq)   -> [B,H,Q,A]
  k,v = iK @ Wkv  -> [B,H,S,A] each
  scores = q k^T / 8, mask -> -1e9, softmax over S
  out = (attn @ v) @ Wo -> [B,Q,D]

Sharding: 8 cores = 2 batches x 4 head-groups (4 heads each).
Each core computes a partial [Q, D] = ctx_local @ Wo_rows(local heads);
host sums the 4 partials per batch (row-parallel Wo unshard).

Mask pruning: masked s positions contribute exactly 0 to the softmax
(exp(-1e9) == 0.0 in f32, same as the reference), so the host gathers
only the unmasked iK rows (~S/2 of them), padded to a multiple of 128
with zero K columns and -1e9 bias so padding also exps to exactly 0.

Device layout trick: everything is computed "transposed" (feature dim on
partitions) so no on-device transposes are needed:
  - host ships iQ^T, iK^T (pre-tiled [128, 8, n])
  - qT[a,q], kT[a,s] from matmul(lhsT=W, rhs=iX^T)
  - scoresT[s,q] = matmul(lhsT=kT_slice, rhs=qT)       (K=64 contraction)
  - exp via scalar activation, mask bias is a per-partition bias AP
  - V kept natural [s,a] with an appended ones column -> attn@V matmul
    also yields the softmax denominator row for free
  - ctxT normalized via reciprocal + K=1 outer-product broadcast
  - out[q,n] = matmul(lhsT=ctxT_h tile, rhs=Wo_h rows), accum over heads
All psum->sbuf copies run on DVE so the Scalar engine only does exp
(exp paces the attention phase; the PE p-state depends on keeping up).
"""

import sys
import numpy as np

for _p in ("/opt/trn_rl_repo",):
    if _p not in sys.path:
        sys.path.insert(0, _p)

import ml_dtypes

B, Q, S, D = 2, 2048, 2048, 1024
H, A = 16, 64
HG = 4            # heads per core
NCORES = 8
NEG = -1e9
MIN_NST = 9       # S tiles after mask pruning (1152 slots; count ~1024)

_cache = {}


def _build_program(nst):
    import concourse.bass as bass  # noqa
    import concourse.bacc as bacc
    import concourse.tile as tile
    from concourse import mybir

    f32 = mybir.dt.float32
    bf16 = mybir.dt.bfloat16
    EXP = mybir.ActivationFunctionType.Exp
    LN = mybir.ActivationFunctionType.Ln
    MULT = mybir.AluOpType.mult

    nc = bacc.Bacc("TRN2", target_bir_lowering=False, debug=False)

    SP = nst * 128  # padded kept-S extent
    iqt = nc.dram_tensor("iqt", [128, 8, Q], bf16, kind="ExternalInput").ap()
    ikt = nc.dram_tensor("ikt", [128, 8, SP], bf16, kind="ExternalInput").ap()
    wq = nc.dram_tensor("wq", [128, 8, 256], bf16, kind="ExternalInput").ap()
    wk = nc.dram_tensor("wk", [128, 8, 256], bf16, kind="ExternalInput").ap()
    wv = nc.dram_tensor("wv", [128, 8, 256], bf16, kind="ExternalInput").ap()
    wo = nc.dram_tensor("wo", [128, 2, D], bf16, kind="ExternalInput").ap()
    mb = nc.dram_tensor("mb", [128, nst], f32, kind="ExternalInput").ap()
    out = nc.dram_tensor("out", [128, 16, D], f32, kind="ExternalOutput").ap()

    NQT = Q // 128          # 16 q tiles
    NDT = D // 128          # 8 d tiles

    with tile.TileContext(nc) as tc:
        with (
            tc.tile_pool(name="persist", bufs=1) as persist,
            tc.tile_pool(name="expp", bufs=4) as expp,
            tc.tile_pool(name="outp", bufs=3) as outp,
            tc.tile_pool(name="srp", bufs=4) as srp,
            tc.tile_pool(name="scp", bufs=2, space="PSUM") as scp,
            tc.tile_pool(name="ctxp", bufs=1, space="PSUM") as ctxp,
            tc.tile_pool(name="bcp", bufs=1, space="PSUM") as bcp,
        ):
            # ---- persistent loads, in consumption order: small weights
            # first, then iK^T/iQ^T tiles interleaved so the KT projection
            # starts as soon as wk + the first ikt tile land.
            wk_sb = persist.tile([128, 8, 256], bf16, tag="wk")
            nc.sync.dma_start(wk_sb[:], wk[:])
            wq_sb = persist.tile([128, 8, 256], bf16, tag="wq")
            nc.sync.dma_start(wq_sb[:], wq[:])
            wv_sb = persist.tile([128, 8, 256], bf16, tag="wv")
            nc.sync.dma_start(wv_sb[:], wv[:])
            wo_sb = persist.tile([128, 2, D], bf16, tag="wo")
            nc.sync.dma_start(wo_sb[:], wo[:])
            mb_sb = persist.tile([128, nst], f32, tag="mb")
            nc.sync.dma_start(mb_sb[:], mb[:])
            iqt_sb = persist.tile([128, 8, Q], bf16, tag="iqt")
            ikt_sb = persist.tile([128, 8, SP], bf16, tag="ikt")
            for dt_i in range(8):
                hk = SP // 2
                nc.sync.dma_start(ikt_sb[:, dt_i, :hk], ikt[:, dt_i, :hk])
                nc.sync.dma_start(ikt_sb[:, dt_i, hk:], ikt[:, dt_i, hk:])
                nc.sync.dma_start(iqt_sb[:, dt_i, :1024], iqt[:, dt_i, :1024])
                nc.sync.dma_start(iqt_sb[:, dt_i, 1024:], iqt[:, dt_i, 1024:])

            qt_sb = persist.tile([128, 2, Q], bf16, tag="qt")
            kt_sb = persist.tile([128, 2, SP], bf16, tag="kt")
            # V padded to 128 cols (fast weight load wants full-width lhsT);
            # col 64 = ones (softmax denominator row), cols 65.. = zeros.
            v_sb = persist.tile([128, nst, HG, 128], bf16, tag="v")
            nc.vector.memset(v_sb[:], 0.0)
            nc.vector.memset(v_sb[:, :, :, 64:65], 1.0)
            # two heads packed per 128-partition tile for the Wo matmul;
            # separate tiles per q-chunk so Wo reads of chunk 0 don't
            # falsely depend on chunk 1 writes.
            ctxn = [
                [
                    persist.tile(
                        [128, 1024], bf16, tag=f"ctxn{qc}{t}", name=f"ctxn{qc}{t}"
                    )
                    for t in range(2)
                ]
                for qc in range(2)
            ]

            # ---- projections: kT [a, s] first (its inputs land first) ----
            for wsb, xsb, osb, nf in (
                (wk_sb, ikt_sb, kt_sb, SP),
                (wq_sb, iqt_sb, qt_sb, Q),
            ):
                nchunk = (nf + 1023) // 1024
                for at in range(2):          # 128-wide slab of the 256 head cols
                    for qc in range(nchunk):  # 1024-wide output chunk
                        w = min(1024, nf - qc * 1024)
                        ps = scp.tile([128, 1024], mybir.dt.float32, tag="mm")
                        for c in range(0, w, 512):
                            cw = min(512, w - c)
                            for dt_i in range(NDT):
                                nc.tensor.matmul(
                                    ps[:, c:c + cw],
                                    lhsT=wsb[:, dt_i, at * 128:(at + 1) * 128],
                                    rhs=xsb[:, dt_i, qc * 1024 + c:qc * 1024 + c + cw],
                                    start=(dt_i == 0),
                                    stop=(dt_i == NDT - 1),
                                )
                        nc.vector.tensor_copy(
                            out=osb[:, at, qc * 1024:qc * 1024 + w], in_=ps[:, :w]
                        )

            # ---- V projection: natural [s, a] per head (+ ones col kept) ----
            for st in range(nst):
                ps = scp.tile([128, HG, 64], mybir.dt.float32, tag="mm")
                for dt_i in range(NDT):
                    nc.tensor.matmul(
                        ps[:],
                        lhsT=ikt_sb[:, dt_i, st * 128:(st + 1) * 128],
                        rhs=wv_sb[:, dt_i, :],
                        start=(dt_i == 0),
                        stop=(dt_i == NDT - 1),
                    )
                nc.vector.tensor_copy(out=v_sb[:, st, :, 0:64], in_=ps[:])

            # ---- attention per (q-chunk, head) + interleaved Wo ----
            def emit_wo(qt):
                ps = scp.tile([128, 1024], mybir.dt.float32, tag="mm")
                for c in range(2):
                    for t in range(2):
                        nc.tensor.matmul(
                            ps[:, c * 512:(c + 1) * 512],
                            lhsT=ctxn[qt // 8][t][:, (qt % 8) * 128:
                                                  (qt % 8 + 1) * 128],
                            rhs=wo_sb[:, t, c * 512:(c + 1) * 512],
                            start=(t == 0),
                            stop=(t == 1),
                        )
                ob = outp.tile([128, 1024], mybir.dt.float32, tag="ob")
                nc.vector.tensor_copy(out=ob[:], in_=ps[:])
                nc.sync.dma_start(out[:, qt, :], ob[:])

            for qc in range(2):
                q0 = qc * 1024
                for h in range(HG):
                    po = (h % 2) * 64
                    ti = h // 2
                    ctx = ctxp.tile([128, 1024], mybir.dt.float32, tag="ctx")
                    for st in range(nst):
                        sc = scp.tile([128, 1024], mybir.dt.float32, tag="mm")
                        for c in range(2):
                            nc.tensor.matmul(
                                sc[:, c * 512:(c + 1) * 512],
                                lhsT=kt_sb[po:po + 64, ti, st * 128:(st + 1) * 128],
                                rhs=qt_sb[po:po + 64, ti,
                                          q0 + c * 512:q0 + (c + 1) * 512],
                                start=True,
                                stop=True,
                            )
                        ex = expp.tile([128, 1024], bf16, tag="exp")
                        nc.scalar.activation(
                            out=ex[:], in_=sc[:], func=EXP,
                            bias=mb_sb[:, st:st + 1], scale=0.125,
                        )
                        for c in range(2):
                            nc.tensor.matmul(
                                ctx[:, c * 512:(c + 1) * 512],
                                lhsT=v_sb[:, st, h, :],
                                rhs=ex[:, c * 512:(c + 1) * 512],
                                start=(st == 0),
                                stop=(st == nst - 1),
                            )
                    # Copy ctx out of PSUM on ACT right after this head's
                    # last exp (hidden behind the next head's scores), so
                    # the PSUM bank frees immediately. The reciprocal runs
                    # on DVE (slow but fully overlapped), broadcast on
                    # GpSimd, multiply on DVE into the packed ctxn tile.
                    ctxu = srp.tile([65, 1024], mybir.dt.float32, tag="ctxu")
                    nc.scalar.copy(out=ctxu[:], in_=ctx[:65, :])
                    recip = srp.tile([1, 1024], mybir.dt.float32, tag="recip")
                    if qc == 1 and h == HG - 1:
                        # last head: ln->exp on ACT (1.5us) instead of the
                        # 6.5us DVE reciprocal, to shorten the kernel tail
                        lnd = srp.tile([1, 1024], mybir.dt.float32, tag="lnd")
                        nc.scalar.activation(out=lnd[:], in_=ctxu[64:65, :],
                                             func=LN)
                        nc.scalar.activation(out=recip[:], in_=lnd[:],
                                             func=EXP, scale=-1.0)
                    else:
                        nc.vector.reciprocal(recip[:], ctxu[64:65, :])
                    bcd = srp.tile([64, 1024], mybir.dt.float32, tag="bcd")
                    nc.gpsimd.partition_broadcast(bcd[:], recip[:])
                    nc.vector.tensor_tensor(
                        ctxn[qc][ti][po:po + 64, :],
                        ctxu[0:64, :], bcd[:], MULT,
                    )
                    # interleave the previous q-chunk's Wo tiles as PE filler
                    if qc == 1:
                        for qt in (h * 2, h * 2 + 1):
                            emit_wo(qt)
            for qt in range(8, 16):
                emit_wo(qt)

    nc.compile()
    return nc


def _get_program(nst):
    if nst not in _cache:
        _cache[nst] = _build_program(nst)
    return _cache[nst]


def _prep_inputs(iQ, iK, mask, Wq, Wkv, Wo):
    """Build the 8 per-core input maps (host-side shard + prune + cast)."""
    bf = ml_dtypes.bfloat16
    iQ = np.asarray(iQ, dtype=np.float32)
    iK = np.asarray(iK, dtype=np.float32)
    mask = np.asarray(mask)
    Wq = np.asarray(Wq, dtype=np.float32)
    Wkv = np.asarray(Wkv, dtype=np.float32)
    Wo = np.asarray(Wo, dtype=np.float32)

    def tile_kxn(a):  # [K=1024, N] -> [128, K/128, N]
        K, N = a.shape
        return np.ascontiguousarray(
            a.reshape(K // 128, 128, N).transpose(1, 0, 2)
        )

    kept = [np.flatnonzero(~mask[b, 0]) for b in range(B)]
    nst = max(MIN_NST, max((len(k) + 127) // 128 for k in kept))
    SP = nst * 128

    per_b = {}
    for b in range(B):
        nk = len(kept[b])
        ikt_full = np.zeros((1024, SP), dtype=np.float32)
        ikt_full[:, :nk] = iK[b][kept[b], :].T
        bias = np.full(SP, np.float32(NEG), dtype=np.float32)
        bias[:nk] = 0.0
        per_b[b] = {
            "iqt": tile_kxn(iQ[b].T).astype(bf),
            "ikt": tile_kxn(ikt_full).astype(bf),
            "mb": np.ascontiguousarray(bias.reshape(nst, 128).T),
        }
    in_maps = []
    for c in range(NCORES):
        b, g = divmod(c, NCORES // B)
        cols = slice(g * 256, (g + 1) * 256)
        wo_g = Wo[g * 256:(g + 1) * 256, :]          # [256, 1024]
        in_maps.append({
            "iqt": per_b[b]["iqt"],
            "ikt": per_b[b]["ikt"],
            "mb": per_b[b]["mb"],
            "wq": tile_kxn(Wq[:, cols]).astype(bf),
            "wk": tile_kxn(Wkv[:, cols]).astype(bf),
            "wv": tile_kxn(Wkv[:, 1024 + g * 256:1024 + (g + 1) * 256]).astype(bf),
            "wo": np.ascontiguousarray(
                wo_g.reshape(2, 128, D).transpose(1, 0, 2)
            ).astype(bf),
        })
    return in_maps, nst


def _run(inputs, trace=False):
    from concourse.bass_utils import run_bass_kernel_spmd

    in_maps, nst = _prep_inputs(**inputs)
    nc = _get_program(nst)
    res = run_bass_kernel_spmd(
        nc, in_maps, list(range(NCORES)), trace=trace
    )
    outs = []
    for b in range(B):
        acc = None
        for g in range(NCORES // B):
            o = np.asarray(
                res.results[b * (NCORES // B) + g]["out"], dtype=np.float32
            )
            acc = o if acc is None else acc + o
        # [128, 16, 1024] -> [2048, 1024]
        outs.append(acc.transpose(1, 0, 2).reshape(Q, D))
    return np.stack(outs), res


def kernel(**inputs):
    out, _ = _run(inputs, trace=False)
    return out
